# revision 63
# baseline (speedup 1.0000x reference)
"""Trainium2 Bass kernel for nn_Attention (B=4, N=2048, C=1024, H=16).

Sharding: 8 cores; core c -> (batch b = c//2, head-group g = c%2 of 8 heads).
Data-parallel on B, tensor-parallel on H.  Each core computes a full-shape
[C, N] (transposed) partial of the output projection for its head slice; the
host transposes, sums the two partials per batch and adds proj_b.

v6 (from the v5 trace: PE 262us active of 313us, full clock mid-kernel,
losses concentrated at startup 17us / qp boundary 8us / tail 16us):

  - DMA issue spread across the SP + Act HWDGE queues and the gpsimd
    SWDGE, ordered by first compute use; wk/wq stored m-major so the
    first kT/qT units' weights are the first 256KB chunk.  All KC*2
    exp-bias tiles are resident in SBUF (no qp-boundary bias stall).
  - PE warmup: dummy matmuls on the first-landing tile bridge the
    DMA-bound startup so the HAM clock gate reaches 2.4GHz by ~13us.
  - Attention per (q-block, head): ST scores -> ACT exp -> DVE mul by
    exp(bias) -> PV accumulate; pv banks evacuated on DVE (ACT is the
    body's pacing engine); 1/rowsum via DMA-spread + parallel DVE
    reciprocal + DRAM-bounce broadcast + gpsimd multiply.
  - Dense-GEMM filler (next kT/qT pair, qp0 projection) interleaved into
    the attention slots, placed early/mid-slot so slot-boundary consumers
    never wait on a late evacuation.
  - Last slot (qp1, t3) runs as two 512-column sub-slots.  Tail-critical
    normalizes ride the idle sync queue; the final head defers its
    normalize to a single-partition DVE reciprocal + PE rank-1 broadcast
    (no DMA bounce), hidden under the qs=2 projection units; qs=3 chains
    finish with t4=3 last and outputs drain on both HWDGE queues.

Mask compaction: keys permuted per batch so unmasked keys come first; only
the first KU (= roundup128(max unmasked count)) keys kept.  Dropped keys are
masked and contribute exactly 0 in the reference too.
"""
import os
import sys

sys.path.insert(0, "/opt/trn_rl_repo")

import numpy as np
import ml_dtypes
from contextlib import ExitStack

import concourse.bass as bass
import concourse.bacc as bacc
import concourse.tile as tile
from concourse import mybir
from concourse.bass_utils import run_bass_kernel_spmd

F32 = mybir.dt.float32
F32R = mybir.dt.float32r
BF16 = mybir.dt.bfloat16
AF = mybir.ActivationFunctionType
NPBF = ml_dtypes.bfloat16

B, N, C, H, D = 4, 2048, 1024, 16, 64
HG = 8            # heads per core
CG = HG * D       # 512: per-core c_out slice of q/k/v and of proj input
P = 128
E = D + 2         # 66: v columns + ones column + pad (4B-aligned bf16 slices)
MASK_VALUE = -65504.0
SCALE = float(D) ** -0.5

_prog_cache = {}


def _ceil_div(a, b):
    return (a + b - 1) // b


def _build(KU, use_qb):
    """Build the SPMD Bass program (same on all 8 cores) for KU kept keys."""
    KC = KU // P               # number of 128-token key chunks
    QB = N // 512              # 4 query blocks of 512

    nc = bacc.Bacc("TRN2", target_bir_lowering=False, debug=False, num_devices=8)
    xT_d = nc.declare_dram_parameter("xT", [C, N], BF16, isOutput=False)
    xpT_d = nc.declare_dram_parameter("xpT", [C, KU], BF16, isOutput=False)
    expb_d = nc.declare_dram_parameter("expbT", [KU, N], BF16, isOutput=False)
    wq_d = nc.declare_dram_parameter("wq", [P, 8 * CG], BF16, isOutput=False)
    wk_d = nc.declare_dram_parameter("wk", [P, 8 * CG], BF16, isOutput=False)
    wv_d = nc.declare_dram_parameter("wv", [P, 8 * CG], BF16, isOutput=False)
    wp_d = nc.declare_dram_parameter("wp", [P, 4 * C], BF16, isOutput=False)
    qb_d = nc.declare_dram_parameter("qb", [CG], F32, isOutput=False)
    vb_d = nc.declare_dram_parameter("vb", [1, CG], F32, isOutput=False)
    ones_d = nc.declare_dram_parameter("ones", [1, P], F32, isOutput=False)
    vones_d = nc.declare_dram_parameter("vones", [P, HG * E], BF16, isOutput=False)
    outp_d = nc.declare_dram_parameter("outp", [C, N], BF16, isOutput=True)

    scr_d = nc.dram_tensor("rs_scratch", [20, 1024], F32)

    with ExitStack() as ctx:
        tc = ctx.enter_context(tile.TileContext(nc))
        persist = ctx.enter_context(tc.tile_pool(name="persist", bufs=1))
        const = ctx.enter_context(tc.tile_pool(name="const", bufs=1))

        ones1 = const.tile([1, P], F32R, name="ones1")
        vb_t = const.tile([1, CG], F32R, name="vb_t")
        qb_t = const.tile([P, 4], F32, name="qb_t")
        vo_t = const.tile([P, HG * E], BF16, name="vo_t")
        onesb = const.tile([1, P], BF16, name="onesb")

        qTt = [persist.tile([P, N], BF16, name=f"qT{i}") for i in range(4)]
        kTt = [persist.tile([P, KU], BF16, name=f"kT{i}") for i in range(4)]
        vat = [persist.tile([P, HG * E], BF16, name=f"va{i}") for i in range(KC)]
        ott = [persist.tile([P, N], BF16, name=f"ot{i}") for i in range(4)]
        wp_t = persist.tile([P, 4 * C], BF16, name="wp_t")
        wq_t = persist.tile([P, 8 * CG], BF16, name="wq_t")
        wk_t = persist.tile([P, 8 * CG], BF16, name="wk_t")
        wv_t = persist.tile([P, 8 * CG], BF16, name="wv_t")
        xf = [persist.tile([P, N], BF16, name=f"xf{k}") for k in range(8)]
        xp = [persist.tile([P, KU], BF16, name=f"xp{k}") for k in range(8)]

        kblks = [(b0, min(512, KU - b0)) for b0 in range(0, KU, 512)]

        # ---- DMA issue: SP + Act HWDGE queues (+ gpsimd SWDGE for the qp1
        # bias tiles), ordered by first compute use.  wk/wq are laid out
        # m-major on the host so the m=0 slices (first kT/qT units) are the
        # first 256KB chunk. ----
        # sync(SP): warm tile first (PE warmup fodder), then wk m0 -> xp lo
        # -> xp hi (kT blocks 1/2 + v) -> wk m1-3
        warm = const.tile([P, 512], BF16, name="warm")
        nc.sync.dma_start(warm[:], xT_d[0:P, 0:512])
        nc.sync.dma_start(wk_t[:, 0:1024], wk_d[:, 0:1024])
        for k in range(8):
            nc.sync.dma_start(xp[k][:, 0:512], xpT_d[k * P : (k + 1) * P, 0:512])
        for k in range(8):
            nc.sync.dma_start(
                xp[k][:, 512:KU], xpT_d[k * P : (k + 1) * P, 512:KU]
            )
        nc.sync.dma_start(wk_t[:, 1024:4096], wk_d[:, 1024:4096])
        # scalar(Act): v deps first (vb/ones gate the v-chunk matmul chain),
        # then wq m0 (qT dep) and xf lo (qT qp0 dep)
        nc.scalar.dma_start(ones1[:], ones_d[:].bitcast(F32R))
        nc.scalar.dma_start(vb_t[:], vb_d[:].bitcast(F32R))
        nc.scalar.dma_start(wv_t[:], wv_d[:])
        nc.scalar.dma_start(vo_t[:], vones_d[:])
        nc.scalar.dma_start(wq_t[:, 0:1024], wq_d[:, 0:1024])
        if use_qb:
            nc.scalar.dma_start(
                qb_t[:], qb_d[:].rearrange("(m p) -> p m", p=P)
            )
        for k in range(8):
            nc.scalar.dma_start(
                xf[k][:, 0:1024], xT_d[k * P : (k + 1) * P, 0:1024]
            )
        # bf16 ones row for the tail's rank-1 broadcast (gpsimd DMA casts)
        nc.gpsimd.dma_start(onesb[:], ones_d[:])

        with tc.tile_pool(name="bsb", bufs=2 * KC) as bpool, tc.tile_pool(
            name="pp", bufs=4
        ) as ppool, tc.tile_pool(name="ovp", bufs=3) as ovpool, tc.tile_pool(
            name="rsp", bufs=4
        ) as rpool, tc.tile_pool(
            name="oev2", bufs=3
        ) as oev2, tc.tile_pool(
            name="bcp", bufs=2
        ) as bcpool, tc.tile_pool(
            name="pst", bufs=2, space="PSUM"
        ) as pst, tc.tile_pool(
            name="ppv", bufs=2, space="PSUM"
        ) as ppv, tc.tile_pool(
            name="fps", bufs=2, space="PSUM"
        ) as fps:

            # ---- all exp-bias tiles resident: qp0 split sync/scalar (first
            # chunks via scalar so they land before the sync queue drains
            # xp+xf), qp1 on gpsimd SWDGE ----
            btiles = [[None] * KC for _ in range(2)]
            for kc in range(KC):
                bt = bpool.tile([P, 1024], BF16, name="b_t", tag="bt")
                eng = nc.scalar if kc < 3 else nc.sync
                eng.dma_start(bt[:], expb_d[kc * P : (kc + 1) * P, 0:1024])
                btiles[0][kc] = bt
            # late consumers: wq m1-3 (qp0 t0 fillers), xf hi, wp
            nc.scalar.dma_start(wq_t[:, 1024:4096], wq_d[:, 1024:4096])
            for k in range(8):
                nc.sync.dma_start(
                    xf[k][:, 1024:N], xT_d[k * P : (k + 1) * P, 1024:N]
                )
            nc.sync.dma_start(wp_t[:], wp_d[:])
            for kc in range(KC):
                bt = bpool.tile([P, 1024], BF16, name="b_t2", tag="bt")
                nc.gpsimd.dma_start(bt[:], expb_d[kc * P : (kc + 1) * P, 1024:N])
                btiles[1][kc] = bt

            # ---- emit units (each ~1.7-2.1us of PE work through fps) ----
            def emit_kT_block(m, i):
                b0, w = kblks[i]
                ps = fps.tile([P, 512], F32, name="ps_k", tag="fps")
                for kc8 in range(8):
                    lw = wk_t[:, m * 1024 + kc8 * P : m * 1024 + (kc8 + 1) * P]
                    nc.tensor.matmul(
                        ps[:, :w],
                        lhsT=lw,
                        rhs=xp[kc8][:, b0 : b0 + w],
                        start=(kc8 == 0),
                        stop=(kc8 == 7),
                    )
                nc.vector.tensor_copy(kTt[m][:, b0 : b0 + w], ps[:, :w])

            def emit_qT_block(m, nb):
                ps = fps.tile([P, 512], F32, name="ps_q", tag="fps")
                for kc8 in range(8):
                    lw = wq_t[:, m * 1024 + kc8 * P : m * 1024 + (kc8 + 1) * P]
                    nc.tensor.matmul(
                        ps[:],
                        lhsT=lw,
                        rhs=xf[kc8][:, nb * 512 : (nb + 1) * 512],
                        start=(kc8 == 0),
                        stop=(kc8 == 7),
                    )
                if use_qb:
                    nc.scalar.activation(
                        qTt[m][:, nb * 512 : (nb + 1) * 512],
                        ps[:],
                        AF.Identity,
                        bias=qb_t[:, m : m + 1],
                    )
                else:
                    nc.vector.tensor_copy(
                        qTt[m][:, nb * 512 : (nb + 1) * 512], ps[:]
                    )

            def emit_v_chunk(tm):
                psv = fps.tile([P, CG], F32, name="ps_v", tag="fps")
                for kc8 in range(8):
                    nc.tensor.matmul(
                        psv[:],
                        lhsT=xp[kc8][:, tm * P : (tm + 1) * P],
                        rhs=wv_t[:, kc8 * CG : (kc8 + 1) * CG],
                        start=(kc8 == 0),
                        stop=False,
                    )
                nc.tensor.matmul(
                    psv[:], lhsT=ones1[0:1, :], rhs=vb_t[0:1, :], start=False,
                    stop=True,
                )
                nc.vector.tensor_copy(vat[tm][:], vo_t[:])
                nc.vector.tensor_copy(
                    vat[tm][:].rearrange("p (h e) -> p h e", e=E)[:, :, 0:D],
                    psv[:].rearrange("p (h e) -> p h e", e=D),
                )

            def emit_proj_cq(cm, qs, oeng=None, pool=None):
                pl, tag = pool or (fps, "fps")
                ps = pl.tile([P, 512], F32, name="ps_p", tag=tag)
                for t4 in range(4):
                    lw = wp_t[:, t4 * C + cm * P : t4 * C + (cm + 1) * P]
                    nc.tensor.matmul(
                        ps[:],
                        lhsT=lw,
                        rhs=ott[t4][:, qs * 512 : (qs + 1) * 512],
                        start=(t4 == 0),
                        stop=(t4 == 3),
                    )
                osb = oev2.tile([P, 512], BF16, name="o_sb", tag="osb")
                nc.scalar.activation(osb[:], ps[:], AF.Copy)
                (oeng or nc.gpsimd).dma_start(
                    outp_d[cm * P : (cm + 1) * P, qs * 512 : (qs + 1) * 512],
                    osb[:],
                )

            def emit_dummy():
                # keeps the HAM activity window busy; no consumers.  Streams
                # the warm tile, which is the first DMA to land.
                ps = fps.tile([P, 512], F32, name="ps_d", tag="fps")
                nc.tensor.matmul(
                    ps[:], lhsT=warm[:, 0:P], rhs=warm[:],
                    start=True, stop=True,
                )

            norm_it = [0]

            def normalize(t, po, cols_lo, width, ov, ov_off, norm):
                """1/rowsum spread across partitions by DMA for a parallel
                DVE reciprocal, DRAM-bounce stride-0 broadcast, multiply on
                the otherwise-idle GPSIMD.  norm='sync' (tail heads) routes
                the bounce through the idle sync HWDGE queue and multiplies
                on DVE so nothing queues behind mid-kernel pool work."""
                it = norm_it[0]
                norm_it[0] += 1
                wp8 = width // P
                deng = nc.sync if norm == "sync" else nc.gpsimd
                rsw = rpool.tile([P, 8], F32, name="rsw_t", tag="rsw")
                nc.sync.dma_start(
                    rsw[:, 0:wp8], ov[D : D + 1, ov_off : ov_off + width]
                )
                rsw2 = rpool.tile([P, 8], F32, name="rsw2_t", tag="rsw2")
                nc.vector.reciprocal(rsw2[:, 0:wp8], rsw[:, 0:wp8])
                deng.dma_start(scr_d[it : it + 1, 0:width], rsw2[:, 0:wp8])
                bcs = bcpool.tile([D, 1024], F32, name="bcs_t", tag="bcs")
                row = scr_d[it : it + 1, :]
                deng.dma_start(
                    bcs[:, 0:width],
                    bass.AP(
                        tensor=row.tensor,
                        offset=row.offset,
                        ap=[[0, D], [1, width]],
                    ),
                )
                meng = nc.vector if norm == "sync" else nc.gpsimd
                meng.tensor_mul(
                    ott[t][po : po + D, cols_lo : cols_lo + width],
                    ov[0:D, ov_off : ov_off + width],
                    bcs[:, 0:width],
                )

            # ---- PE warmup: dummy matmuls streaming the first-landing tile
            # bridge the initial DMA wait so the HAM clock gate ramps to
            # 2.4GHz before real work arrives ----
            for _ in range(28):
                ps = fps.tile([P, 512], F32, name="ps_w", tag="fps")
                nc.tensor.matmul(
                    ps[:], lhsT=warm[:, 0:P], rhs=warm[:],
                    start=True, stop=True,
                )

            # ---- pre-attention GEMMs (kT/qT pair 0 + all v), emission
            # ordered to match DMA arrival (xp-lo, wv | xp-hi, xf-lo) ----
            emit_kT_block(0, 0)
            for _ in range(3):
                emit_dummy()
            for tm in range(min(4, KC)):
                emit_v_chunk(tm)
                emit_dummy()
                emit_dummy()
            for i in range(1, len(kblks)):
                emit_kT_block(0, i)
                emit_dummy()
                emit_dummy()
            for tm in range(4, KC):
                emit_v_chunk(tm)
                emit_dummy()
                emit_dummy()
            emit_qT_block(0, 0)
            emit_dummy()
            emit_qT_block(0, 1)
            emit_dummy()

            # ---- filler schedule: slot (qp, t) -> list of thunks ----
            # qp1 proj of qp0's columns: 16 units spread over t0..t2 (t3 is
            # the split sub-slot tail).
            qp1_proj_units = [(qs, cm) for cm in range(8) for qs in range(2)]
            qp1_slot_units = [
                qp1_proj_units[0:4],
                qp1_proj_units[4:8],
                qp1_proj_units[8:12],
            ]
            qp1_sub0_units = qp1_proj_units[12:16]

            def filler_for(qp, t):
                """Returns [(step, thunk)] — fillers placed early/mid slot so
                their consumers (next slot's scores) never wait on the DVE
                evacuation at a slot boundary."""
                th = []
                if qp == 0:
                    if t < 3:
                        m = t + 1
                        steps = [0, 3, 7]
                        for i in range(len(kblks)):
                            th.append(
                                (steps[i], lambda m=m, i=i: emit_kT_block(m, i))
                            )
                        th.append((10, lambda m=m: emit_qT_block(m, 0)))
                        th.append((13, lambda m=m: emit_qT_block(m, 1)))
                    if t == 3:
                        # needed right at (qp1, t0)
                        th.append((0, lambda: emit_qT_block(0, 2)))
                        th.append((5, lambda: emit_qT_block(0, 3)))
                        th.append((10, emit_dummy))
                        th.append((14, emit_dummy))
                else:
                    if t < 3:
                        # qT(t+1) qp1-half, needed at (qp1, t+1)
                        th.append((0, lambda m=t + 1: emit_qT_block(m, 2)))
                        th.append((3, lambda m=t + 1: emit_qT_block(m, 3)))
                        for i, (qs, cm) in enumerate(qp1_slot_units[t]):
                            th.append(
                                (6 + 3 * i,
                                 lambda cm=cm, qs=qs: emit_proj_cq(cm, qs))
                            )
                    if t == 2:
                        # bridge the t2 -> t3 transition so the HAM window
                        # never sees idle entering the sub-slots
                        th.append((16, emit_dummy))
                        th.append((17, emit_dummy))
                return th

            def attn_head(qp, t, hh, q0, width, qcol_off, btq, filler_sched,
                          step0, norm):
                """One head over `width` query columns starting at q0+qcol_off.
                Returns next step counter."""
                h = 2 * t + hh
                po = hh * D
                nj = width // 512
                pvh = [
                    ppv.tile([P, 512], F32, name="pv_t", tag="pv")
                    for _ in range(nj)
                ]
                step = step0
                for kc in range(KC):
                    stt = pst.tile([P, width], F32, name="st_t", tag="stt")
                    lw = kTt[t][po : po + D, kc * P : (kc + 1) * P]
                    for j in range(nj):
                        nc.tensor.matmul(
                            stt[:, j * 512 : (j + 1) * 512],
                            lhsT=lw,
                            rhs=qTt[t][
                                po : po + D,
                                q0 + qcol_off + j * 512 : q0 + qcol_off + (j + 1) * 512,
                            ],
                            start=True,
                            stop=True,
                        )
                    pt = ppool.tile([P, width], BF16, name="p_t", tag="pt")
                    nc.scalar.activation(pt[:], stt[:], AF.Exp)
                    nc.vector.tensor_mul(
                        pt[:], pt[:], btq[kc][:, qcol_off : qcol_off + width]
                    )
                    if kc == 0:
                        # the first PV of a head waits on the previous pv
                        # evacuation: run filler ahead of it so the PE queue
                        # isn't head-of-line blocked
                        for fn in filler_sched.get(step, []):
                            fn()
                    lv = vat[kc][:, h * E : (h + 1) * E]
                    for j in range(nj):
                        nc.tensor.matmul(
                            pvh[j][0:E, :],
                            lhsT=lv,
                            rhs=pt[:, j * 512 : (j + 1) * 512],
                            start=(kc == 0),
                            stop=(kc == KC - 1),
                        )
                    if kc != 0:
                        for fn in filler_sched.get(step, []):
                            fn()
                    step += 1
                # evacuate pv fast (on DVE: ACT's exp backlog would delay
                # the bank release and stall the next head's first PV),
                # normalize from the SBUF copy
                ov = ovpool.tile([P, 1024], F32, name="ov_t", tag="ov")
                nc.vector.tensor_copy(ov[0:E, 0:512], pvh[0][0:E, :])
                if nj == 2:
                    # both halves on DVE: ACT is the body's pacing engine
                    # (exp stream), so keep the copy off it
                    nc.vector.tensor_copy(ov[0:E, 512:1024], pvh[1][0:E, :])
                if norm == "defer":
                    return step, ov
                normalize(t, po, q0 + qcol_off, width, ov, 0, norm)
                return step, None

            for qp in range(QB // 2):
                q0 = qp * 1024
                btq = btiles[qp]
                for t in range(4):
                    if qp == 1 and t == 3:
                        break
                    sched = {}
                    for stp, fn in filler_for(qp, t):
                        sched.setdefault(min(stp, 2 * KC - 1), []).append(fn)
                    step = 0
                    for hh in range(2):
                        step, _ = attn_head(
                            qp, t, hh, q0, 1024, 0, btq, sched, step, "pool"
                        )

            # ---- (qp1, t3): two 512-col sub-slots.  The qs=2 units (cols
            # 1024:1536, gated by sub0's last normalize) run right after
            # sub1's attention while the final head's deferred normalize
            # (ACT reciprocal + PE rank-1 broadcast, no DMA bounce) lands;
            # then the qs=3 chains finish with t4=3 last. ----
            q0 = 1024
            btq = btiles[1]
            ov_last = None
            for sub in range(2):
                sched = {}
                steps_units = [
                    (st, u)
                    for st, u in zip(
                        (0, 5) if sub == 0 else (2, 7),
                        qp1_sub0_units[sub * 2 : sub * 2 + 2],
                    )
                ]
                for stp, (qs, cm) in steps_units:
                    sched.setdefault(min(stp, 2 * KC - 1), []).append(
                        lambda cm=cm, qs=qs: emit_proj_cq(cm, qs)
                    )
                if sub == 0:
                    sched.setdefault(11, []).append(emit_dummy)
                step = 0
                for hh in range(2):
                    norm = "sync"
                    if sub == 1 and hh == 1:
                        norm = "defer"
                    step, ovh = attn_head(
                        1, 3, hh, q0, 512, sub * 512, btq, sched, step, norm
                    )
                    if ovh is not None:
                        ov_last = ovh
            # deferred normalize for the last head: single-partition DVE
            # reciprocal (~3us) hidden under the qs=2 projection units
            rro = bcpool.tile([1, 512], BF16, name="rro_t", tag="bcs")
            with nc.allow_low_precision(reason="1/rowsum row in bf16 for the "
                                        "rank-1 broadcast matmul"):
                nc.vector.reciprocal(rro[0:1, :], ov_last[D : D + 1, 0:512])
            # qs=2 units: their gate (sub0-hh1's sync-bounce) lands while
            # sub1's attention runs; they in turn cover the reciprocal.
            # Rotate PSUM pools (all free by now) so the ACT-copy recycle
            # latency of a 2-slot pool doesn't pace the units.
            tailpools = [(fps, "fps"), (pst, "stt")]
            for cm in range(8):
                emit_proj_cq(
                    cm, 2, oeng=(nc.sync if cm % 2 == 0 else nc.scalar),
                    pool=tailpools[cm % 2],
                )
            bps = fps.tile([P, 512], F32, name="ps_b", tag="fps")
            nc.tensor.matmul(
                bps[0:D, :], lhsT=onesb[0:1, 0:D], rhs=rro[0:1, :],
                start=True, stop=True,
            )
            nc.vector.tensor_mul(
                ott[3][D : 2 * D, 1536:2048], ov_last[0:D, 0:512], bps[0:D, :]
            )
            # tail: qs=3 units; outputs split across the two idle HWDGE
            # queues so the final drain isn't serialized on one queue
            for cm in range(8):
                emit_proj_cq(
                    cm, 3, oeng=(nc.sync if cm % 2 == 0 else nc.scalar),
                    pool=tailpools[cm % 2],
                )
    nc.finalize()
    return nc


def kernel(
    x=None,
    attention_mask=None,
    attention_bias=None,
    qkv_w=None,
    q_bias=None,
    v_bias=None,
    proj_w=None,
    proj_b=None,
):
    x = np.ascontiguousarray(np.asarray(x, dtype=np.float32))
    mask = np.asarray(attention_mask).astype(bool)
    bias = np.asarray(attention_bias, dtype=np.float32)
    qkv_w = np.asarray(qkv_w, dtype=np.float32)
    q_bias = np.asarray(q_bias, dtype=np.float32)
    v_bias = np.asarray(v_bias, dtype=np.float32)
    proj_w = np.asarray(proj_w, dtype=np.float32)
    proj_b = np.asarray(proj_b, dtype=np.float32)

    assert x.shape == (B, N, C), x.shape

    # --- mask compaction: unmasked keys first, keep KU of them ---
    perms, us = [], []
    for b in range(B):
        perms.append(np.argsort(mask[b], kind="stable"))
        us.append(int((~mask[b]).sum()))
    KU = min(N, max(P, _ceil_div(max(us), P) * P))
    use_qb = bool(np.any(q_bias))

    key = (KU, use_qb)
    if key not in _prog_cache:
        _prog_cache[key] = _build(KU, use_qb)
    nc = _prog_cache[key]

    ones_h = np.ones((1, P), dtype=np.float32)
    vones_h = np.zeros((P, HG * E), dtype=NPBF)
    vones_h.reshape(P, HG, E)[:, :, D] = 1.0
    mv = np.float32(MASK_VALUE)

    per_b = []
    for b in range(B):
        perm = perms[b][:KU]
        xT = np.ascontiguousarray(x[b].T.astype(NPBF))
        xpT = np.ascontiguousarray(x[b][perm].T.astype(NPBF))
        biasT = bias[b].T[perm] + np.where(mask[b][perm], mv, np.float32(0.0))[:, None]
        expbT = np.ascontiguousarray(np.exp(biasT, dtype=np.float32).astype(NPBF))
        per_b.append((xT, xpT, expbT))

    per_g = []
    for g in range(2):
        sl = slice(g * CG, (g + 1) * CG)

        def tile_w(wT, ncols):  # [C_in, ncols] -> [128, (C_in//128)*ncols]
            return np.ascontiguousarray(
                wT.reshape(wT.shape[0] // P, P, ncols)
                .transpose(1, 0, 2)
                .reshape(P, -1)
                .astype(NPBF)
            )

        def tile_w_mm(wT):  # m-major: [1024, 512] -> [128, (m)(kc8)(128)]
            return np.ascontiguousarray(
                wT.reshape(8, P, 4, P)
                .transpose(1, 2, 0, 3)
                .reshape(P, -1)
                .astype(NPBF)
            )

        wq = tile_w_mm((qkv_w[sl, :] * np.float32(SCALE)).T.astype(np.float32))
        wk = tile_w_mm(
            np.ascontiguousarray(qkv_w[C + g * CG : C + (g + 1) * CG, :].T).astype(
                np.float32
            )
        )
        wv = tile_w(
            np.ascontiguousarray(qkv_w[2 * C + g * CG : 2 * C + (g + 1) * CG, :].T), CG
        )
        wp = tile_w(np.ascontiguousarray(proj_w[:, sl].T), C)
        qb = np.ascontiguousarray(q_bias[sl] * np.float32(SCALE))
        vb = np.ascontiguousarray(v_bias[sl][None, :])
        per_g.append((wq, wk, wv, wp, qb, vb))

    in_maps = []
    for c in range(8):
        b, g = c // 2, c % 2
        xT, xpT, expbT = per_b[b]
        wq, wk, wv, wp, qb, vb = per_g[g]
        in_maps.append(
            {
                "xT": xT,
                "xpT": xpT,
                "expbT": expbT,
                "wq": wq,
                "wk": wk,
                "wv": wv,
                "wp": wp,
                "qb": qb,
                "vb": vb,
                "ones": ones_h,
                "vones": vones_h,
            }
        )

    trace = bool(int(os.environ.get("KBENCH_TRACE", "0")))
    kw = {}
    if trace:
        kw = dict(
            trace=True,
            trace_cores=[
                int(t) for t in os.environ.get("KBENCH_TRACE_CORES", "0").split(",")
            ],
        )
    res = run_bass_kernel_spmd(nc, in_maps, list(range(8)), **kw)
    if trace:
        kernel.last_exec_ns = res.exec_time_ns
        kernel.last_result = res

    out = np.empty((B, N, C), dtype=np.float32)
    for b in range(B):
        outT = res.results[2 * b]["outp"].astype(np.float32) + res.results[
            2 * b + 1
        ]["outp"].astype(np.float32)
        out[b] = outT.T
        out[b] += proj_b[None, :]
    return out


kernel.last_exec_ns = None
kernel.last_result = None


# revision 65
# speedup vs baseline: 1.0040x; 1.0040x over previous
"""Trainium2 Bass kernel for nn_Attention (B=4, N=2048, C=1024, H=16).

Sharding: 8 cores; core c -> (batch b = c//2, head-group g = c%2 of 8 heads).
Data-parallel on B, tensor-parallel on H.  Each core computes a full-shape
[C, N] (transposed) partial of the output projection for its head slice; the
host transposes, sums the two partials per batch and adds proj_b.

v6 (from the v5 trace: PE 262us active of 313us, full clock mid-kernel,
losses concentrated at startup 17us / qp boundary 8us / tail 16us):

  - DMA issue spread across the SP + Act HWDGE queues and the gpsimd
    SWDGE, ordered by first compute use; wk/wq stored m-major so the
    first kT/qT units' weights are the first 256KB chunk.  All KC*2
    exp-bias tiles are resident in SBUF (no qp-boundary bias stall).
  - PE warmup: dummy matmuls on the first-landing tile bridge the
    DMA-bound startup so the HAM clock gate reaches 2.4GHz by ~13us.
  - Attention per (q-block, head): ST scores -> ACT exp -> DVE mul by
    exp(bias) -> PV accumulate; pv banks evacuated on DVE (ACT is the
    body's pacing engine); 1/rowsum via DMA-spread + parallel DVE
    reciprocal + DRAM-bounce broadcast + gpsimd multiply.
  - Dense-GEMM filler (next kT/qT pair, qp0 projection) interleaved into
    the attention slots, placed early/mid-slot so slot-boundary consumers
    never wait on a late evacuation.
  - Last slot (qp1, t3) runs as two 512-column sub-slots.  Tail-critical
    normalizes ride the idle sync queue; the final head defers its
    normalize to a single-partition DVE reciprocal + PE rank-1 broadcast
    (no DMA bounce), hidden under the qs=2 projection units; qs=3 chains
    finish with t4=3 last and outputs drain on both HWDGE queues.

Mask compaction: keys permuted per batch so unmasked keys come first; only
the first KU (= roundup128(max unmasked count)) keys kept.  Dropped keys are
masked and contribute exactly 0 in the reference too.
"""
import os
import sys

sys.path.insert(0, "/opt/trn_rl_repo")

import numpy as np
import ml_dtypes
from contextlib import ExitStack

import concourse.bass as bass
import concourse.bacc as bacc
import concourse.tile as tile
from concourse import mybir
from concourse.bass_utils import run_bass_kernel_spmd

F32 = mybir.dt.float32
F32R = mybir.dt.float32r
BF16 = mybir.dt.bfloat16
AF = mybir.ActivationFunctionType
NPBF = ml_dtypes.bfloat16

B, N, C, H, D = 4, 2048, 1024, 16, 64
HG = 8            # heads per core
CG = HG * D       # 512: per-core c_out slice of q/k/v and of proj input
P = 128
E = D + 2         # 66: v columns + ones column + pad (4B-aligned bf16 slices)
MASK_VALUE = -65504.0
SCALE = float(D) ** -0.5

_prog_cache = {}


def _ceil_div(a, b):
    return (a + b - 1) // b


def _build(KU, use_qb):
    """Build the SPMD Bass program (same on all 8 cores) for KU kept keys."""
    KC = KU // P               # number of 128-token key chunks
    QB = N // 512              # 4 query blocks of 512

    nc = bacc.Bacc("TRN2", target_bir_lowering=False, debug=False, num_devices=8)
    xT_d = nc.declare_dram_parameter("xT", [C, N], BF16, isOutput=False)
    xpT_d = nc.declare_dram_parameter("xpT", [C, KU], BF16, isOutput=False)
    expb_d = nc.declare_dram_parameter("expbT", [KU, N], BF16, isOutput=False)
    wq_d = nc.declare_dram_parameter("wq", [P, 8 * CG], BF16, isOutput=False)
    wk_d = nc.declare_dram_parameter("wk", [P, 8 * CG], BF16, isOutput=False)
    wv_d = nc.declare_dram_parameter("wv", [P, 8 * CG], BF16, isOutput=False)
    wp_d = nc.declare_dram_parameter("wp", [P, 4 * C], BF16, isOutput=False)
    qb_d = nc.declare_dram_parameter("qb", [CG], F32, isOutput=False)
    vb_d = nc.declare_dram_parameter("vb", [1, CG], F32, isOutput=False)
    ones_d = nc.declare_dram_parameter("ones", [1, P], F32, isOutput=False)
    vones_d = nc.declare_dram_parameter("vones", [P, HG * E], BF16, isOutput=False)
    outp_d = nc.declare_dram_parameter("outp", [C, N], BF16, isOutput=True)

    scr_d = nc.dram_tensor("rs_scratch", [20, 1024], F32)

    with ExitStack() as ctx:
        tc = ctx.enter_context(tile.TileContext(nc))
        persist = ctx.enter_context(tc.tile_pool(name="persist", bufs=1))
        const = ctx.enter_context(tc.tile_pool(name="const", bufs=1))

        ones1 = const.tile([1, P], F32R, name="ones1")
        vb_t = const.tile([1, CG], F32R, name="vb_t")
        qb_t = const.tile([P, 4], F32, name="qb_t")
        vo_t = const.tile([P, HG * E], BF16, name="vo_t")
        onesb = const.tile([1, P], BF16, name="onesb")

        qTt = [persist.tile([P, N], BF16, name=f"qT{i}") for i in range(4)]
        kTt = [persist.tile([P, KU], BF16, name=f"kT{i}") for i in range(4)]
        vat = [persist.tile([P, HG * E], BF16, name=f"va{i}") for i in range(KC)]
        ott = [persist.tile([P, N], BF16, name=f"ot{i}") for i in range(4)]
        wp_t = persist.tile([P, 4 * C], BF16, name="wp_t")
        wq_t = persist.tile([P, 8 * CG], BF16, name="wq_t")
        wk_t = persist.tile([P, 8 * CG], BF16, name="wk_t")
        wv_t = persist.tile([P, 8 * CG], BF16, name="wv_t")
        xf = [persist.tile([P, N], BF16, name=f"xf{k}") for k in range(8)]
        xp = [persist.tile([P, KU], BF16, name=f"xp{k}") for k in range(8)]

        kblks = [(b0, min(512, KU - b0)) for b0 in range(0, KU, 512)]

        # ---- DMA issue: SP + Act HWDGE queues (+ gpsimd SWDGE for the qp1
        # bias tiles), ordered by first compute use.  wk/wq are laid out
        # m-major on the host so the m=0 slices (first kT/qT units) are the
        # first 256KB chunk. ----
        # warm tile via memset — no DMA dependency, so PE warmup dummies can
        # start within ~1us of kernel entry and hold the HAM clock up
        # through the DMA-bound startup
        warm = const.tile([P, 512], BF16, name="warm")
        nc.gpsimd.memset(warm[:], 1.0)
        # sync(SP): wk m0 -> xp lo -> xp hi (kT blocks 1/2 + v) -> wk m1-3
        nc.sync.dma_start(wk_t[:, 0:1024], wk_d[:, 0:1024])
        for k in range(8):
            nc.sync.dma_start(xp[k][:, 0:512], xpT_d[k * P : (k + 1) * P, 0:512])
        for k in range(8):
            nc.sync.dma_start(
                xp[k][:, 512:KU], xpT_d[k * P : (k + 1) * P, 512:KU]
            )
        nc.sync.dma_start(wk_t[:, 1024:4096], wk_d[:, 1024:4096])
        # scalar(Act): v deps first (vb/ones gate the v-chunk matmul chain),
        # then wq m0 (qT dep) and xf lo (qT qp0 dep)
        nc.scalar.dma_start(ones1[:], ones_d[:].bitcast(F32R))
        nc.scalar.dma_start(vb_t[:], vb_d[:].bitcast(F32R))
        nc.scalar.dma_start(wv_t[:], wv_d[:])
        nc.scalar.dma_start(vo_t[:], vones_d[:])
        nc.scalar.dma_start(wq_t[:, 0:1024], wq_d[:, 0:1024])
        if use_qb:
            nc.scalar.dma_start(
                qb_t[:], qb_d[:].rearrange("(m p) -> p m", p=P)
            )
        for k in range(8):
            nc.scalar.dma_start(
                xf[k][:, 0:1024], xT_d[k * P : (k + 1) * P, 0:1024]
            )
        # bf16 ones row for the tail's rank-1 broadcast (gpsimd DMA casts)
        nc.gpsimd.dma_start(onesb[:], ones_d[:])

        with tc.tile_pool(name="bsb", bufs=2 * KC) as bpool, tc.tile_pool(
            name="pp", bufs=4
        ) as ppool, tc.tile_pool(name="ovp", bufs=3) as ovpool, tc.tile_pool(
            name="rsp", bufs=4
        ) as rpool, tc.tile_pool(
            name="oev2", bufs=3
        ) as oev2, tc.tile_pool(
            name="bcp", bufs=2
        ) as bcpool, tc.tile_pool(
            name="pst", bufs=2, space="PSUM"
        ) as pst, tc.tile_pool(
            name="ppv", bufs=2, space="PSUM"
        ) as ppv, tc.tile_pool(
            name="fps", bufs=2, space="PSUM"
        ) as fps:

            # ---- all exp-bias tiles resident: qp0 split sync/scalar (first
            # chunks via scalar so they land before the sync queue drains
            # xp+xf), qp1 on gpsimd SWDGE ----
            btiles = [[None] * KC for _ in range(2)]
            for kc in range(KC):
                bt = bpool.tile([P, 1024], BF16, name="b_t", tag="bt")
                eng = nc.scalar if kc < 3 else nc.sync
                eng.dma_start(bt[:], expb_d[kc * P : (kc + 1) * P, 0:1024])
                btiles[0][kc] = bt
            # late consumers: wq m1-3 (qp0 t0 fillers), xf hi, wp
            nc.scalar.dma_start(wq_t[:, 1024:4096], wq_d[:, 1024:4096])
            for k in range(8):
                nc.sync.dma_start(
                    xf[k][:, 1024:N], xT_d[k * P : (k + 1) * P, 1024:N]
                )
            nc.sync.dma_start(wp_t[:], wp_d[:])
            for kc in range(KC):
                bt = bpool.tile([P, 1024], BF16, name="b_t2", tag="bt")
                nc.gpsimd.dma_start(bt[:], expb_d[kc * P : (kc + 1) * P, 1024:N])
                btiles[1][kc] = bt

            # ---- emit units (each ~1.7-2.1us of PE work through fps) ----
            def emit_kT_block(m, i):
                b0, w = kblks[i]
                ps = fps.tile([P, 512], F32, name="ps_k", tag="fps")
                for kc8 in range(8):
                    lw = wk_t[:, m * 1024 + kc8 * P : m * 1024 + (kc8 + 1) * P]
                    nc.tensor.matmul(
                        ps[:, :w],
                        lhsT=lw,
                        rhs=xp[kc8][:, b0 : b0 + w],
                        start=(kc8 == 0),
                        stop=(kc8 == 7),
                    )
                nc.vector.tensor_copy(kTt[m][:, b0 : b0 + w], ps[:, :w])

            def emit_qT_block(m, nb):
                ps = fps.tile([P, 512], F32, name="ps_q", tag="fps")
                for kc8 in range(8):
                    lw = wq_t[:, m * 1024 + kc8 * P : m * 1024 + (kc8 + 1) * P]
                    nc.tensor.matmul(
                        ps[:],
                        lhsT=lw,
                        rhs=xf[kc8][:, nb * 512 : (nb + 1) * 512],
                        start=(kc8 == 0),
                        stop=(kc8 == 7),
                    )
                if use_qb:
                    nc.scalar.activation(
                        qTt[m][:, nb * 512 : (nb + 1) * 512],
                        ps[:],
                        AF.Identity,
                        bias=qb_t[:, m : m + 1],
                    )
                else:
                    nc.vector.tensor_copy(
                        qTt[m][:, nb * 512 : (nb + 1) * 512], ps[:]
                    )

            def emit_v_chunk(tm):
                psv = fps.tile([P, CG], F32, name="ps_v", tag="fps")
                for kc8 in range(8):
                    nc.tensor.matmul(
                        psv[:],
                        lhsT=xp[kc8][:, tm * P : (tm + 1) * P],
                        rhs=wv_t[:, kc8 * CG : (kc8 + 1) * CG],
                        start=(kc8 == 0),
                        stop=False,
                    )
                nc.tensor.matmul(
                    psv[:], lhsT=ones1[0:1, :], rhs=vb_t[0:1, :], start=False,
                    stop=True,
                )
                nc.vector.tensor_copy(vat[tm][:], vo_t[:])
                nc.vector.tensor_copy(
                    vat[tm][:].rearrange("p (h e) -> p h e", e=E)[:, :, 0:D],
                    psv[:].rearrange("p (h e) -> p h e", e=D),
                )

            def emit_proj_cq(cm, qs, oeng=None, pool=None):
                pl, tag = pool or (fps, "fps")
                ps = pl.tile([P, 512], F32, name="ps_p", tag=tag)
                for t4 in range(4):
                    lw = wp_t[:, t4 * C + cm * P : t4 * C + (cm + 1) * P]
                    nc.tensor.matmul(
                        ps[:],
                        lhsT=lw,
                        rhs=ott[t4][:, qs * 512 : (qs + 1) * 512],
                        start=(t4 == 0),
                        stop=(t4 == 3),
                    )
                osb = oev2.tile([P, 512], BF16, name="o_sb", tag="osb")
                nc.scalar.activation(osb[:], ps[:], AF.Copy)
                (oeng or nc.gpsimd).dma_start(
                    outp_d[cm * P : (cm + 1) * P, qs * 512 : (qs + 1) * 512],
                    osb[:],
                )

            def emit_dummy():
                # keeps the HAM activity window busy; no consumers.  Streams
                # the warm tile, which is the first DMA to land.
                ps = fps.tile([P, 512], F32, name="ps_d", tag="fps")
                nc.tensor.matmul(
                    ps[:], lhsT=warm[:, 0:P], rhs=warm[:],
                    start=True, stop=True,
                )

            norm_it = [0]

            def normalize(t, po, cols_lo, width, ov, ov_off, norm):
                """1/rowsum spread across partitions by DMA for a parallel
                DVE reciprocal, DRAM-bounce stride-0 broadcast, multiply on
                the otherwise-idle GPSIMD.  norm='sync' (tail heads) routes
                the bounce through the idle sync HWDGE queue and multiplies
                on DVE so nothing queues behind mid-kernel pool work."""
                it = norm_it[0]
                norm_it[0] += 1
                wp8 = width // P
                deng = nc.sync if norm == "sync" else nc.gpsimd
                rsw = rpool.tile([P, 8], F32, name="rsw_t", tag="rsw")
                nc.sync.dma_start(
                    rsw[:, 0:wp8], ov[D : D + 1, ov_off : ov_off + width]
                )
                rsw2 = rpool.tile([P, 8], F32, name="rsw2_t", tag="rsw2")
                nc.vector.reciprocal(rsw2[:, 0:wp8], rsw[:, 0:wp8])
                deng.dma_start(scr_d[it : it + 1, 0:width], rsw2[:, 0:wp8])
                bcs = bcpool.tile([D, 1024], F32, name="bcs_t", tag="bcs")
                row = scr_d[it : it + 1, :]
                deng.dma_start(
                    bcs[:, 0:width],
                    bass.AP(
                        tensor=row.tensor,
                        offset=row.offset,
                        ap=[[0, D], [1, width]],
                    ),
                )
                meng = nc.vector if norm == "sync" else nc.gpsimd
                meng.tensor_mul(
                    ott[t][po : po + D, cols_lo : cols_lo + width],
                    ov[0:D, ov_off : ov_off + width],
                    bcs[:, 0:width],
                )

            # ---- PE warmup: dummy matmuls streaming the first-landing tile
            # bridge the initial DMA wait so the HAM clock gate ramps to
            # 2.4GHz before real work arrives ----
            for _ in range(40):
                ps = fps.tile([P, 512], F32, name="ps_w", tag="fps")
                nc.tensor.matmul(
                    ps[:], lhsT=warm[:, 0:P], rhs=warm[:],
                    start=True, stop=True,
                )

            # ---- pre-attention GEMMs (kT/qT pair 0 + all v), emission
            # ordered to match DMA arrival (xp-lo, wv | xp-hi, xf-lo) ----
            emit_kT_block(0, 0)
            for _ in range(3):
                emit_dummy()
            for tm in range(min(4, KC)):
                emit_v_chunk(tm)
                emit_dummy()
                emit_dummy()
            for i in range(1, len(kblks)):
                emit_kT_block(0, i)
                emit_dummy()
                emit_dummy()
            for tm in range(4, KC):
                emit_v_chunk(tm)
                emit_dummy()
                emit_dummy()
            emit_qT_block(0, 0)
            emit_dummy()
            emit_qT_block(0, 1)
            emit_dummy()

            # ---- filler schedule: slot (qp, t) -> list of thunks ----
            # qp1 proj of qp0's columns: 16 units spread over t0..t2 (t3 is
            # the split sub-slot tail).
            qp1_proj_units = [(qs, cm) for cm in range(8) for qs in range(2)]
            qp1_slot_units = [
                qp1_proj_units[0:4],
                qp1_proj_units[4:8],
                qp1_proj_units[8:12],
            ]
            qp1_sub0_units = qp1_proj_units[12:16]

            def filler_for(qp, t):
                """Returns [(step, thunk)] — fillers placed early/mid slot so
                their consumers (next slot's scores) never wait on the DVE
                evacuation at a slot boundary."""
                th = []
                if qp == 0:
                    if t < 3:
                        m = t + 1
                        steps = [0, 3, 7]
                        for i in range(len(kblks)):
                            th.append(
                                (steps[i], lambda m=m, i=i: emit_kT_block(m, i))
                            )
                        th.append((10, lambda m=m: emit_qT_block(m, 0)))
                        th.append((13, lambda m=m: emit_qT_block(m, 1)))
                    if t == 3:
                        # needed right at (qp1, t0)
                        th.append((0, lambda: emit_qT_block(0, 2)))
                        th.append((5, lambda: emit_qT_block(0, 3)))
                        th.append((10, emit_dummy))
                        th.append((14, emit_dummy))
                else:
                    if t < 3:
                        # qT(t+1) qp1-half, needed at (qp1, t+1)
                        th.append((0, lambda m=t + 1: emit_qT_block(m, 2)))
                        th.append((3, lambda m=t + 1: emit_qT_block(m, 3)))
                        for i, (qs, cm) in enumerate(qp1_slot_units[t]):
                            th.append(
                                (6 + 3 * i,
                                 lambda cm=cm, qs=qs: emit_proj_cq(cm, qs))
                            )
                    if t == 2:
                        # bridge the t2 -> t3 transition so the HAM window
                        # never sees idle entering the sub-slots
                        th.append((16, emit_dummy))
                        th.append((17, emit_dummy))
                return th

            def attn_head(qp, t, hh, q0, width, qcol_off, btq, filler_sched,
                          step0, norm):
                """One head over `width` query columns starting at q0+qcol_off.
                Returns next step counter."""
                h = 2 * t + hh
                po = hh * D
                nj = width // 512
                pvh = [
                    ppv.tile([P, 512], F32, name="pv_t", tag="pv")
                    for _ in range(nj)
                ]
                step = step0
                for kc in range(KC):
                    stt = pst.tile([P, width], F32, name="st_t", tag="stt")
                    lw = kTt[t][po : po + D, kc * P : (kc + 1) * P]
                    for j in range(nj):
                        nc.tensor.matmul(
                            stt[:, j * 512 : (j + 1) * 512],
                            lhsT=lw,
                            rhs=qTt[t][
                                po : po + D,
                                q0 + qcol_off + j * 512 : q0 + qcol_off + (j + 1) * 512,
                            ],
                            start=True,
                            stop=True,
                        )
                    pt = ppool.tile([P, width], BF16, name="p_t", tag="pt")
                    nc.scalar.activation(pt[:], stt[:], AF.Exp)
                    nc.vector.tensor_mul(
                        pt[:], pt[:], btq[kc][:, qcol_off : qcol_off + width]
                    )
                    if kc == 0:
                        # the first PV of a head waits on the previous pv
                        # evacuation: run filler ahead of it so the PE queue
                        # isn't head-of-line blocked
                        for fn in filler_sched.get(step, []):
                            fn()
                    lv = vat[kc][:, h * E : (h + 1) * E]
                    for j in range(nj):
                        nc.tensor.matmul(
                            pvh[j][0:E, :],
                            lhsT=lv,
                            rhs=pt[:, j * 512 : (j + 1) * 512],
                            start=(kc == 0),
                            stop=(kc == KC - 1),
                        )
                    if kc != 0:
                        for fn in filler_sched.get(step, []):
                            fn()
                    step += 1
                # evacuate pv fast (on DVE: ACT's exp backlog would delay
                # the bank release and stall the next head's first PV),
                # normalize from the SBUF copy
                ov = ovpool.tile([P, 1024], F32, name="ov_t", tag="ov")
                nc.vector.tensor_copy(ov[0:E, 0:512], pvh[0][0:E, :])
                if nj == 2:
                    # both halves on DVE: ACT is the body's pacing engine
                    # (exp stream), so keep the copy off it
                    nc.vector.tensor_copy(ov[0:E, 512:1024], pvh[1][0:E, :])
                if norm == "defer":
                    return step, ov
                normalize(t, po, q0 + qcol_off, width, ov, 0, norm)
                return step, None

            for qp in range(QB // 2):
                q0 = qp * 1024
                btq = btiles[qp]
                for t in range(4):
                    if qp == 1 and t == 3:
                        break
                    sched = {}
                    for stp, fn in filler_for(qp, t):
                        sched.setdefault(min(stp, 2 * KC - 1), []).append(fn)
                    step = 0
                    for hh in range(2):
                        step, _ = attn_head(
                            qp, t, hh, q0, 1024, 0, btq, sched, step, "pool"
                        )

            # ---- (qp1, t3): two 512-col sub-slots.  The qs=2 units (cols
            # 1024:1536, gated by sub0's last normalize) run right after
            # sub1's attention while the final head's deferred normalize
            # (ACT reciprocal + PE rank-1 broadcast, no DMA bounce) lands;
            # then the qs=3 chains finish with t4=3 last. ----
            q0 = 1024
            btq = btiles[1]
            ov_last = None
            for sub in range(2):
                sched = {}
                steps_units = [
                    (st, u)
                    for st, u in zip(
                        (0, 5) if sub == 0 else (2, 7),
                        qp1_sub0_units[sub * 2 : sub * 2 + 2],
                    )
                ]
                for stp, (qs, cm) in steps_units:
                    sched.setdefault(min(stp, 2 * KC - 1), []).append(
                        lambda cm=cm, qs=qs: emit_proj_cq(cm, qs)
                    )
                if sub == 0:
                    sched.setdefault(11, []).append(emit_dummy)
                step = 0
                for hh in range(2):
                    norm = "sync"
                    if sub == 1 and hh == 1:
                        norm = "defer"
                    step, ovh = attn_head(
                        1, 3, hh, q0, 512, sub * 512, btq, sched, step, norm
                    )
                    if ovh is not None:
                        ov_last = ovh
            # deferred normalize for the last head: single-partition DVE
            # reciprocal (~3us) hidden under the qs=2 projection units
            rro = bcpool.tile([1, 512], BF16, name="rro_t", tag="bcs")
            with nc.allow_low_precision(reason="1/rowsum row in bf16 for the "
                                        "rank-1 broadcast matmul"):
                nc.vector.reciprocal(rro[0:1, :], ov_last[D : D + 1, 0:512])
            # qs=2 units: their gate (sub0-hh1's sync-bounce) lands while
            # sub1's attention runs; they in turn cover the reciprocal.
            # Rotate PSUM pools (all free by now) so the ACT-copy recycle
            # latency of a 2-slot pool doesn't pace the units.
            tailpools = [(fps, "fps"), (pst, "stt")]
            for cm in range(8):
                emit_proj_cq(
                    cm, 2, oeng=(nc.sync if cm % 2 == 0 else nc.scalar),
                    pool=tailpools[cm % 2],
                )
            bps = fps.tile([P, 512], F32, name="ps_b", tag="fps")
            nc.tensor.matmul(
                bps[0:D, :], lhsT=onesb[0:1, 0:D], rhs=rro[0:1, :],
                start=True, stop=True,
            )
            nc.vector.tensor_mul(
                ott[3][D : 2 * D, 1536:2048], ov_last[0:D, 0:512], bps[0:D, :]
            )
            # tail: qs=3 units; outputs split across the two idle HWDGE
            # queues so the final drain isn't serialized on one queue
            for cm in range(8):
                emit_proj_cq(
                    cm, 3, oeng=(nc.sync if cm % 2 == 0 else nc.scalar),
                    pool=tailpools[cm % 2],
                )
    nc.finalize()
    return nc


def kernel(
    x=None,
    attention_mask=None,
    attention_bias=None,
    qkv_w=None,
    q_bias=None,
    v_bias=None,
    proj_w=None,
    proj_b=None,
):
    x = np.ascontiguousarray(np.asarray(x, dtype=np.float32))
    mask = np.asarray(attention_mask).astype(bool)
    bias = np.asarray(attention_bias, dtype=np.float32)
    qkv_w = np.asarray(qkv_w, dtype=np.float32)
    q_bias = np.asarray(q_bias, dtype=np.float32)
    v_bias = np.asarray(v_bias, dtype=np.float32)
    proj_w = np.asarray(proj_w, dtype=np.float32)
    proj_b = np.asarray(proj_b, dtype=np.float32)

    assert x.shape == (B, N, C), x.shape

    # --- mask compaction: unmasked keys first, keep KU of them ---
    perms, us = [], []
    for b in range(B):
        perms.append(np.argsort(mask[b], kind="stable"))
        us.append(int((~mask[b]).sum()))
    KU = min(N, max(P, _ceil_div(max(us), P) * P))
    use_qb = bool(np.any(q_bias))

    key = (KU, use_qb)
    if key not in _prog_cache:
        _prog_cache[key] = _build(KU, use_qb)
    nc = _prog_cache[key]

    ones_h = np.ones((1, P), dtype=np.float32)
    vones_h = np.zeros((P, HG * E), dtype=NPBF)
    vones_h.reshape(P, HG, E)[:, :, D] = 1.0
    mv = np.float32(MASK_VALUE)

    per_b = []
    for b in range(B):
        perm = perms[b][:KU]
        xT = np.ascontiguousarray(x[b].T.astype(NPBF))
        xpT = np.ascontiguousarray(x[b][perm].T.astype(NPBF))
        biasT = bias[b].T[perm] + np.where(mask[b][perm], mv, np.float32(0.0))[:, None]
        expbT = np.ascontiguousarray(np.exp(biasT, dtype=np.float32).astype(NPBF))
        per_b.append((xT, xpT, expbT))

    per_g = []
    for g in range(2):
        sl = slice(g * CG, (g + 1) * CG)

        def tile_w(wT, ncols):  # [C_in, ncols] -> [128, (C_in//128)*ncols]
            return np.ascontiguousarray(
                wT.reshape(wT.shape[0] // P, P, ncols)
                .transpose(1, 0, 2)
                .reshape(P, -1)
                .astype(NPBF)
            )

        def tile_w_mm(wT):  # m-major: [1024, 512] -> [128, (m)(kc8)(128)]
            return np.ascontiguousarray(
                wT.reshape(8, P, 4, P)
                .transpose(1, 2, 0, 3)
                .reshape(P, -1)
                .astype(NPBF)
            )

        wq = tile_w_mm((qkv_w[sl, :] * np.float32(SCALE)).T.astype(np.float32))
        wk = tile_w_mm(
            np.ascontiguousarray(qkv_w[C + g * CG : C + (g + 1) * CG, :].T).astype(
                np.float32
            )
        )
        wv = tile_w(
            np.ascontiguousarray(qkv_w[2 * C + g * CG : 2 * C + (g + 1) * CG, :].T), CG
        )
        wp = tile_w(np.ascontiguousarray(proj_w[:, sl].T), C)
        qb = np.ascontiguousarray(q_bias[sl] * np.float32(SCALE))
        vb = np.ascontiguousarray(v_bias[sl][None, :])
        per_g.append((wq, wk, wv, wp, qb, vb))

    in_maps = []
    for c in range(8):
        b, g = c // 2, c % 2
        xT, xpT, expbT = per_b[b]
        wq, wk, wv, wp, qb, vb = per_g[g]
        in_maps.append(
            {
                "xT": xT,
                "xpT": xpT,
                "expbT": expbT,
                "wq": wq,
                "wk": wk,
                "wv": wv,
                "wp": wp,
                "qb": qb,
                "vb": vb,
                "ones": ones_h,
                "vones": vones_h,
            }
        )

    trace = bool(int(os.environ.get("KBENCH_TRACE", "0")))
    kw = {}
    if trace:
        kw = dict(
            trace=True,
            trace_cores=[
                int(t) for t in os.environ.get("KBENCH_TRACE_CORES", "0").split(",")
            ],
        )
    res = run_bass_kernel_spmd(nc, in_maps, list(range(8)), **kw)
    if trace:
        kernel.last_exec_ns = res.exec_time_ns
        kernel.last_result = res

    out = np.empty((B, N, C), dtype=np.float32)
    for b in range(B):
        outT = res.results[2 * b]["outp"].astype(np.float32) + res.results[
            2 * b + 1
        ]["outp"].astype(np.float32)
        out[b] = outT.T
        out[b] += proj_b[None, :]
    return out


kernel.last_exec_ns = None
kernel.last_result = None
